# revision 56
# baseline (speedup 1.0000x reference)
"""Qwen3-style GQA attention (B=1, S=2048, DM=2048, H=16, KV=4, D=128) on 8 TRN2 cores.

Sharding: tensor-parallel over heads. Core c computes Q heads {2c, 2c+1} and
KV head c//2 end-to-end, then a partial output hs_part = gated_local @ Wo_rows.
Host sums the 8 partials.

Precision scheme: the q/k score path runs f32r (12-bit host-prerounded
operands, fp32_mode=HIGH 2-pass matmuls) -- fp16 operands measured 2.6e-2
rel err vs the 2e-2 gate, so the score path keeps f32r. The gate/V
projections and the Wo matmul run fp16 single-pass (1 cycle/row, their noise
is linear and ~3e-4); hs streams in both f32r (sync queue) and fp16 (scalar
queue) copies. Post-softmax probabilities, diag(1/Z), and V run bf16.

Activation-table discipline: P1's scalar engine only ever runs Sqrt (rms),
P3's only Exp (softmax + gate sigmoid as 1/(1+exp(-x)) with the reciprocal
on the vector engine; raw gate is stored in P1 and the sigmoid deferred to
P3). Two ACT_TABLE_LOADs total instead of per-iteration thrash.

P1 structure: one fused projection pass streams hsT chunks once; the first
kc chunk of each weight + hs DMAs ahead of everything so matmuls start ~5us
in. RMSNorm + RoPE jobs are deferred one sq slice and emitted as 4
stage-thunks interleaved into the next slice's projection k4 loop, so every
engine's in-order queue sees 3 independent jobs per stage (chains pipeline
instead of serializing). Rotate-half is an exact partition-offset SBUF DMA
on the gpsimd queue; the three sumsq matmuls of a batch share one PSUM bank
at 32-row offsets.

P3 structure: dual-pass softmax per (h, qb) -- a bf16 max pre-pass feeds
exp's bias so the f32r score pass goes matmul->exp with no reduce between
(PSUM banks free immediately). P^T for AV comes from bf16 matmuls against
diag(1/Z), fusing normalization into the transpose. Cross-phase software
pipelining: chunk qc's PuT/AV/Wo work is emitted as thunks drained between
chunk qc+1's softmax chains, keeping the PE warm through the reduce/exp
latency; per-kb AV matmuls are emitted one kb late so the in-order PE queue
never stalls on a puts copy. PSUM evacuations alternate vector/scalar.

The last sq slice's norm+rope stages are deferred into P3 entirely: their
scratch lives in pools that outlive P1 (manual __enter__/__exit__), their
sumsq/broadcast matmuls run in the early-opened ptp PSUM pool, and the four
stage thunks seed qc=0's fill queue so they interleave with the first
softmax chains. This keeps the P1 pool-close barrier (which gates P3's
first PSUM matmuls on the last reader of every closed pool) free of the
norm tail. Note co pool must stay at bufs=2: a single cpo buffer serializes
every Wo output DMA against the next block's copies (+40us measured).

Measured (8 cores, this problem): 332-343us, rel err 1.15e-2.
Rejected variants (measured): all-fp16 q/k (2.6e-2 err), single-pass
PSUM-resident softmax (bank-holding serializes chains, +30us), XBAR
dma_start_transpose for P^T (transfer time stalls AV, +40us) and for V
(+7us, collides with the hh16 stream on the scalar DGE queue), Ln+Exp rms
(table-set thrash, 2.5us/job), single-pass softmax at qc<=1 only (+8us),
fusing the rope add into persistent q_hi (+50us, cause unknown).
"""

import numpy as np

S = 2048
DM = 2048
D = 128
HPC = 2           # q heads per core
NCORES = 8
SCALING = float(D) ** 0.5
EPS = 1e-6
P = 128
KCH = DM // P     # 16 contraction chunks for projections
NQB = S // P      # 16 q blocks
NSC = S // 512    # 4 seq chunks of 512

_cache = {}


def _round_fp32r(x):
    x = np.ascontiguousarray(x, dtype=np.float32)
    b = x.view(np.uint32).astype(np.uint64)
    lsb = (b >> 12) & 1
    r = (b + 0x7FF + lsb) & 0xFFFFF000
    return r.astype(np.uint32).view(np.float32)


def _build_nc():
    import concourse.tile as tile
    from concourse import bacc, mybir

    F32 = mybir.dt.float32
    F32R = mybir.dt.float32r
    F16 = mybir.dt.float16
    BF16 = mybir.dt.bfloat16
    AF = mybir.ActivationFunctionType
    from concourse.alu_op_type import AluOpType as ALU
    AX = mybir.AxisListType.X

    nc = bacc.Bacc(None, target_bir_lowering=False, debug=False)

    with nc.allow_low_precision(reason="f32r/fp16/bf16 operands are a "
                                "deliberate precision/speed tradeoff"), \
         tile.TileContext(nc) as tc:
        with tc.tile_pool(name="dram", bufs=1, space="DRAM") as dram:
            hsT = dram.tile([P, KCH, S], F32R, kind="ExternalInput", name="hsT", uniquify=False)
            hsT16 = dram.tile([P, KCH, S], F16, kind="ExternalInput", name="hsT16", uniquify=False)
            wq = dram.tile([P, KCH, HPC * P], F32R, kind="ExternalInput", name="wq", uniquify=False)
            wk = dram.tile([P, KCH, P], F32R, kind="ExternalInput", name="wk", uniquify=False)
            wg = dram.tile([P, KCH, HPC * P], F16, kind="ExternalInput", name="wg", uniquify=False)
            wv = dram.tile([P, KCH, P], F16, kind="ExternalInput", name="wv", uniquify=False)
            wo = dram.tile([P, HPC, DM], F16, kind="ExternalInput", name="wo", uniquify=False)
            cosT = dram.tile([P, S], F32, kind="ExternalInput", name="cosT", uniquify=False)
            sinTs = dram.tile([P, S], F32, kind="ExternalInput", name="sinTs", uniquify=False)
            wi2q = dram.tile([P, 1], F16, kind="ExternalInput", name="wi2q", uniquify=False)
            wi2k = dram.tile([P, 1], F16, kind="ExternalInput", name="wi2k", uniquify=False)
            identb = dram.tile([P, P], BF16, kind="ExternalInput", name="identb", uniquify=False)
            oner = dram.tile([1, P], F32, kind="ExternalInput", name="oner", uniquify=False)
            triu = dram.tile([P, P], F32, kind="ExternalInput", name="triu", uniquify=False)
            out = dram.tile([S, DM], F32, kind="ExternalOutput", name="out", uniquify=False)

        # persistent SBUF (whole kernel)
        with tc.tile_pool(name="persist", bufs=1) as pers:
            wi2q_sb = pers.tile([P, 1], F16)
            dum = pers.tile([P, 64], F16)
            wi2k_sb = pers.tile([P, 1], F16)
            identb_sb = pers.tile([P, P], BF16)
            oner_sb = pers.tile([1, P], F32)
            triu_sb = pers.tile([P, P], F32)
            eps_sb = pers.tile([4, 1], F32)
            k_hi = pers.tile([P, S], F32R)
            k_hb = pers.tile([P, S], BF16)
            q_hi = pers.tile([P, HPC, S], F32R)
            q_hb = pers.tile([P, HPC, S], BF16)
            graw = pers.tile([P, HPC, S], F32)      # raw gate (sigmoid in P3)
            v_r = pers.tile([P, NQB, P], BF16)      # V untransposed (s-major blocks)
            gated_r = pers.tile([P, HPC, S], F16)

            nc.gpsimd.memset(eps_sb[:], EPS)
            nc.gpsimd.memset(dum[:], 0.0)

            # tail-scratch outlives P1 so the last norm batch's tiles don't
            # extend the P1 pool-close barrier that gates P3's first matmuls
            _tlp_cm = tc.tile_pool(name="tlp", bufs=10)
            _tlr_cm = tc.tile_pool(name="tlr", bufs=1)
            _cs_cm = tc.tile_pool(name="cs", bufs=2)
            _nxr_cm = tc.tile_pool(name="nxr", bufs=5)
            tlp = _tlp_cm.__enter__()
            tlr = _tlr_cm.__enter__()
            cspool = _cs_cm.__enter__()
            nxr = _nxr_cm.__enter__()

            # ====== P1 (fused): all projections + norm + rope + splits ======
            with (
                tc.tile_pool(name="wts", bufs=1) as wpool,
                tc.tile_pool(name="hs1", bufs=2) as hspool,
                tc.tile_pool(name="nsc", bufs=9) as nsc,
                tc.tile_pool(name="rr1", bufs=1) as rr1,
                tc.tile_pool(name="vts", bufs=1) as vtp,
                tc.tile_pool(name="pqk", bufs=1, space="PSUM") as pqk,
                tc.tile_pool(name="pnm", bufs=1, space="PSUM") as pnm,
            ):
                # HAM warm-up: ~5us of dummy matmuls spin the PE during the
                # initial DMA wait; nothing reads the target bank
                dum_ps = pnm.tile([P, 512], F32, tag="psb", name="psb")
                for _ in range(100):
                    nc.tensor.matmul(dum_ps[0:64, 0:64], lhsT=dum[:], rhs=dum[:],
                                     start=True, stop=True)

                wq_sb = wpool.tile([P, KCH, HPC * P], F32R)
                wk_sb = wpool.tile([P, KCH, P], F32R)
                wg_sb = wpool.tile([P, KCH, HPC * P], F16)
                wv_sb = wpool.tile([P, KCH, P], F16)

                pend = []

                def norm_stages(jobs, scp=None, rrp=None, pnp=None):
                    # 4 stage-thunks for up to 3 norm+rope jobs, interleaved
                    # into the next sq's projection emission so each engine's
                    # in-order queue pipelines jobs instead of serializing
                    # behind one job's cross-engine latency chain
                    st = {}
                    sp = scp or nsc
                    rp = rrp or rr1

                    def s1():
                        if pnp is not None:
                            ps3 = pnp.tile([P, 512], F32, name="putp")
                        else:
                            ps3 = pnm.tile([P, 512], F32, tag="ps1", name="ps3")
                        sqfs = []
                        for j, (xr, wvec, xhi, xhb, cos_t, sin_t) in enumerate(jobs):
                            sqf = sp.tile([P, 512], F16, tag="scr", name="sqf")
                            nc.vector.tensor_mul(sqf[:], xr[:], xr[:])
                            sqfs.append(sqf)
                        for j, (xr, wvec, xhi, xhb, cos_t, sin_t) in enumerate(jobs):
                            nc.tensor.matmul(ps3[32 * j:32 * j + 1, :], lhsT=wvec[:],
                                             rhs=sqfs[j][:], start=True, stop=True)
                        st["ps3"] = ps3

                    def s2():
                        rrs = []
                        for j in range(len(jobs)):
                            sqv = rp.tile([1, 512], F32, tag="sqv%d" % (0 if pnp is not None else j), name="sqv")
                            nc.scalar.activation(sqv[:], st["ps3"][32 * j:32 * j + 1, :],
                                                 AF.Sqrt, scale=1.0 / D,
                                                 bias=eps_sb[0:1, :])
                            rr = rp.tile([1, 512], F32, tag="rr%d" % (0 if pnp is not None else j), name="rr")
                            nc.vector.reciprocal_approx_fast(rr[:], sqv[:])
                            rrs.append(rr)
                        st["rrs"] = rrs

                    def s3():
                        xns, rots = [], []
                        for j, (xr, wvec, xhi, xhb, cos_t, sin_t) in enumerate(jobs):
                            if pnp is not None:
                                psb = pnp.tile([P, 512], F32, name="putp")
                            else:
                                psb = pnm.tile([P, 512], F32, tag="psb", name="psb")
                            nc.tensor.matmul(psb[:], lhsT=oner_sb[:],
                                             rhs=st["rrs"][j][:], start=True, stop=True)
                            xn = sp.tile([P, 512], F32, tag="scr", name="xn")
                            nc.vector.tensor_mul(xn[:], xr[:], psb[:])
                            rot = sp.tile([P, 512], F32, tag="scr", name="rot")
                            nc.gpsimd.dma_start(rot[0:64, :], xn[64:128, :])
                            nc.gpsimd.dma_start(rot[64:128, :], xn[0:64, :])
                            xns.append(xn)
                            rots.append(rot)
                        st["xns"], st["rots"] = xns, rots

                    def s4():
                        t2s, t1s = [], []
                        for j, (xr, wvec, xhi, xhb, cos_t, sin_t) in enumerate(jobs):
                            t2 = sp.tile([P, 512], F32, tag="scr", name="t2")
                            nc.vector.tensor_mul(t2[:], st["rots"][j][:], sin_t[:])
                            t1 = sp.tile([P, 512], F32, tag="scr", name="t1")
                            nc.gpsimd.tensor_mul(t1[:], st["xns"][j][:], cos_t[:])
                            t2s.append(t2)
                            t1s.append(t1)
                        for j, (xr, wvec, xhi, xhb, cos_t, sin_t) in enumerate(jobs):
                            xf = sp.tile([P, 512], F32, tag="scr", name="xf")
                            nc.vector.tensor_add(xf[:], t1s[j][:], t2s[j][:])
                            nc.vector.tensor_copy(xhi, xf[:])
                            nc.scalar.copy(xhb, xhi.bitcast(F32))

                    return [s1, s2, s3, s4]

                for sq in range(NSC):
                    stages = norm_stages(pend[:3]) if sq > 0 else None
                    del pend[:3]
                    s0 = sq * 512
                    sl = slice(s0, s0 + 512)
                    cos_t = cspool.tile([P, 512], F32, tag="cos", name="cos_t")
                    sin_t = cspool.tile([P, 512], F32, tag="sin", name="sin_t")
                    if sq > 0:
                        nc.scalar.dma_start(cos_t[:], cosT[:, sl])
                        nc.scalar.dma_start(sin_t[:], sinTs[:, sl])
                    ps_q0 = pqk.tile([P, 512], F32, tag="psq0", name="ps_q0")
                    ps_q1 = pqk.tile([P, 512], F32, tag="psq1", name="ps_q1")
                    ps_k = pqk.tile([P, 512], F32, tag="psk", name="ps_k")
                    ps_g0 = pqk.tile([P, 512], F32, tag="psg0", name="ps_g0")
                    ps_g1 = pqk.tile([P, 512], F32, tag="psg1", name="ps_g1")
                    ps_v = pqk.tile([P, 512], F32, tag="psv", name="ps_v")
                    for k4 in range(4):
                        hh = hspool.tile([P, 4, 512], F32R, tag="hh", name="hh")
                        hh16 = hspool.tile([P, 4, 512], F16, tag="hh16", name="hh16")
                        if sq == 0 and k4 == 0:
                            # critical first chunk: kc=0 of each weight + the
                            # first hs slice, so matmuls start ~5us in; the
                            # persistent constants ride behind them
                            for (dst, srcw) in (
                                (wq_sb, wq), (wk_sb, wk), (wg_sb, wg), (wv_sb, wv),
                            ):
                                nc.sync.dma_start(dst[:, 0:1, :], srcw[:, 0:1, :])
                            nc.sync.dma_start(hh[:, 0:1, :], hsT[:, 0:1, sl])
                            nc.scalar.dma_start(hh16[:, 0:1, :], hsT16[:, 0:1, sl])
                            nc.sync.dma_start(wi2q_sb[:], wi2q[:])
                            nc.sync.dma_start(wi2k_sb[:], wi2k[:])
                            nc.sync.dma_start(identb_sb[:], identb[:])
                            nc.sync.dma_start(oner_sb[:], oner[:])
                            nc.sync.dma_start(triu_sb[:], triu[:])
                            for (dst, srcw) in (
                                (wq_sb, wq), (wk_sb, wk), (wg_sb, wg), (wv_sb, wv),
                            ):
                                nc.sync.dma_start(dst[:, 1:4, :], srcw[:, 1:4, :])
                            nc.sync.dma_start(hh[:, 1:4, :], hsT[:, 1:4, sl])
                            nc.scalar.dma_start(hh16[:, 1:4, :], hsT16[:, 1:4, sl])
                            nc.scalar.dma_start(cos_t[:], cosT[:, sl])
                            nc.scalar.dma_start(sin_t[:], sinTs[:, sl])
                        else:
                            if sq == 0:
                                ksl = slice(k4 * 4, k4 * 4 + 4)
                                for (dst, srcw) in (
                                    (wq_sb, wq), (wk_sb, wk), (wg_sb, wg), (wv_sb, wv),
                                ):
                                    nc.sync.dma_start(dst[:, ksl, :], srcw[:, ksl, :])
                            # fp16 hs copy rides the scalar queue in parallel
                            # with the f32r copy on the sync queue
                            nc.scalar.dma_start(hh16[:], hsT16[:, k4 * 4:k4 * 4 + 4, sl])
                            nc.sync.dma_start(hh[:], hsT[:, k4 * 4:k4 * 4 + 4, sl])
                        for kci in range(4):
                            kc = k4 * 4 + kci
                            st = kc == 0
                            sp = kc == KCH - 1
                            hx = hh[:, kci, :]
                            hx16 = hh16[:, kci, :]
                            nc.tensor.matmul(ps_q0[:], lhsT=wq_sb[:, kc, 0:P],
                                             rhs=hx, start=st, stop=sp)
                            nc.tensor.matmul(ps_q1[:], lhsT=wq_sb[:, kc, P:2 * P],
                                             rhs=hx, start=st, stop=sp)
                            nc.tensor.matmul(ps_k[:], lhsT=wk_sb[:, kc, :],
                                             rhs=hx, start=st, stop=sp)
                            nc.tensor.matmul(ps_g0[:], lhsT=wg_sb[:, kc, 0:P],
                                             rhs=hx16, start=st, stop=sp)
                            nc.tensor.matmul(ps_g1[:], lhsT=wg_sb[:, kc, P:2 * P],
                                             rhs=hx16, start=st, stop=sp)
                            nc.tensor.matmul(ps_v[:], lhsT=wv_sb[:, kc, :],
                                             rhs=hx16, start=st, stop=sp)
                            if sq == 0:
                                # trickle dummies keep the HAM activity window
                                # non-idle through the DMA-paced first slice so
                                # the PE clock never re-throttles
                                for _ in range(8):
                                    nc.tensor.matmul(
                                        dum_ps[0:64, 0:64], lhsT=dum[:],
                                        rhs=dum[:], start=True, stop=True)
                        if stages is not None:
                            stages[k4]()
                    # gate: store raw; sigmoid runs in P3 where exp's table
                    # set is already loaded (copy is in every table set)
                    nc.any.tensor_copy(graw[:, 0, sl], ps_g0[:])
                    nc.any.tensor_copy(graw[:, 1, sl], ps_g1[:])
                    # V: bf16 copy + transpose into s-major blocks
                    vt = vtp.tile([P, 512], BF16, tag="vt", name="vt")
                    nc.any.tensor_copy(vt[:], ps_v[:])
                    for j in range(4):
                        pst = pqk.tile([P, P], BF16, tag="psg0", name="pst")
                        nc.tensor.transpose(pst[:], vt[:, j * P:(j + 1) * P], identb_sb[:])
                        nc.any.tensor_copy(v_r[:, sq * 4 + j, :], pst[:])
                    # Q/K: copy raw projections out now (frees PSUM); the
                    # norm/rope chain is deferred one sq iteration so the next
                    # projection block hides its PE matmuls' input latency
                    for (psd, wvec, xhi, xhb) in (
                        (ps_q0, wi2q_sb, q_hi[:, 0, sl], q_hb[:, 0, sl]),
                        (ps_q1, wi2q_sb, q_hi[:, 1, sl], q_hb[:, 1, sl]),
                        (ps_k, wi2k_sb, k_hi[:, sl], k_hb[:, sl]),
                    ):
                        xr = nxr.tile([P, 512], F32, tag="xr", name="xr")
                        nc.any.tensor_copy(xr[:], psd[:])
                        pend.append((xr, wvec, xhi, xhb, cos_t, sin_t))
                # sq3's norm jobs are deferred into P3's fill queue

            # ====== P3: attention; dual-pass softmax, cross-qc pipeline ======
            _ptp_cm = tc.tile_pool(name="ptp", bufs=2, space="PSUM")
            _otp_cm = tc.tile_pool(name="otp", bufs=1, space="PSUM")
            ptp = _ptp_cm.__enter__()
            otp = _otp_cm.__enter__()
            with (
                tc.tile_pool(name="mxp", bufs=2, space="PSUM") as mxp,
                tc.tile_pool(name="scb", bufs=3, space="PSUM") as scb,
                tc.tile_pool(name="pu", bufs=10) as pupool,
                tc.tile_pool(name="dd", bufs=10) as ddpool,
                tc.tile_pool(name="sm", bufs=16) as smpool,
                tc.tile_pool(name="sgp", bufs=1) as sgpool,
                tc.tile_pool(name="pts", bufs=2) as ptspool,
                tc.tile_pool(name="wop", bufs=1) as wopool,
                tc.tile_pool(name="co", bufs=2) as copool,
            ):
                wo_sb = wopool.tile([P, HPC, DM], F16)
                nc.sync.dma_start(wo_sb[:], wo[:])

                rot3 = [0]

                def evac_copy(dst, src):
                    # alternate PSUM evacuations between vector and scalar
                    # (gpsimd has no PSUM access)
                    r = rot3[0] = (rot3[0] + 1) % 2
                    if r == 0:
                        nc.vector.tensor_copy(dst, src)
                    else:
                        nc.scalar.copy(dst, src)

                def softmax_chain(qc, qbi, h, pu_l, d_l):
                    qb = 4 * qc + qbi
                    r = qb % 4
                    qsl = slice(qb * P, (qb + 1) * P)
                    nful = qc
                    # --- bf16 max pre-pass: approximate row max ---
                    mparts = smpool.tile([P, 8], F32, tag="mp", name="mparts")
                    for kc in range(nful + 1):
                        w = 512 if kc < nful else (r + 1) * P
                        ksl = slice(kc * 512, kc * 512 + w)
                        mx = mxp.tile([P, 512], F32, name="mx")
                        nc.tensor.matmul(mx[:, :w], lhsT=q_hb[:, h, qsl],
                                         rhs=k_hb[:, ksl], start=True, stop=True)
                        if kc == nful:
                            nc.vector.tensor_add(
                                mx[:, r * P:(r + 1) * P],
                                mx[:, r * P:(r + 1) * P], triu_sb[:])
                        nc.vector.tensor_reduce(
                            mparts[:, kc:kc + 1], mx[:, :w], axis=AX, op=ALU.max)
                    negm = smpool.tile([P, 1], F32, tag="negm", name="negm")
                    nc.vector.tensor_reduce(
                        negm[:], mparts[:, :nful + 1], axis=AX, op=ALU.max,
                        negate=True)
                    bias_t = smpool.tile([P, 1], F32, tag="bias", name="bias_t")
                    nc.vector.tensor_scalar_mul(bias_t[:], negm[:], SCALING)
                    # --- f32r scores; exp immediately, no reduce between ---
                    pu = pupool.tile([P, S], BF16, tag="pu", name="pu")
                    zparts = smpool.tile([P, 8], F32, tag="zp", name="zparts")
                    for kc in range(nful + 1):
                        w = 512 if kc < nful else (r + 1) * P
                        ksl = slice(kc * 512, kc * 512 + w)
                        ps = scb.tile([P, 512], F32, name="ps")
                        nc.tensor.matmul(
                            ps[:, :w], lhsT=q_hi[:, h, qsl], rhs=k_hi[:, ksl],
                            start=True, stop=True)
                        if kc == nful:
                            nc.vector.tensor_add(
                                ps[:, r * P:(r + 1) * P],
                                ps[:, r * P:(r + 1) * P], triu_sb[:])
                        nc.scalar.activation(
                            pu[:, kc * 512:kc * 512 + w], ps[:, :w], AF.Exp,
                            scale=SCALING, bias=bias_t[:],
                            accum_out=zparts[:, kc:kc + 1])
                    zsum = smpool.tile([P, 1], F32, tag="zs", name="zsum")
                    nc.vector.tensor_reduce(
                        zsum[:], zparts[:, :nful + 1], axis=AX, op=ALU.add)
                    rz = smpool.tile([P, 1], F32, tag="rz", name="rz")
                    nc.vector.reciprocal_approx_fast(rz[:], zsum[:])
                    dmat = ddpool.tile([P, P], BF16, tag="dm", name="dmat")
                    nc.vector.tensor_scalar_mul(dmat[:], identb_sb[:], rz[:])
                    pu_l[(h, qb)] = pu
                    d_l[(h, qb)] = dmat

                def avwo_thunks(qc, pu_l, d_l):
                    # PuT+AV per (h, kb) with the AV matmul deferred one kb so
                    # the in-order PE queue never waits on a puts copy; then
                    # the Wo partials + output DMA for this q-chunk's rows.
                    kmax = 4 * qc + 3
                    thunks = []
                    for h in range(HPC):
                        st8 = {"prev": None, "ot": None}

                        def mk_kb(h, kb, st8):
                            def t():
                                if st8["ot"] is None:
                                    st8["ot"] = otp.tile([P, 512], F32, name="ot_ps")
                                putp = ptp.tile([P, 512], F32, name="putp")
                                i0 = max(kb - 4 * qc, 0)
                                for j in range(i0, 4):
                                    qb = 4 * qc + j
                                    nc.tensor.matmul(
                                        putp[:, j * P:(j + 1) * P],
                                        lhsT=pu_l[(h, qb)][:, kb * P:(kb + 1) * P],
                                        rhs=d_l[(h, qb)][:],
                                        start=True, stop=True)
                                puts = ptspool.tile([P, 512], BF16, name="puts")
                                evac_copy(puts[:, i0 * P:], putp[:, i0 * P:])
                                if st8["prev"] is not None:
                                    pkb, pputs, pi0 = st8["prev"]
                                    nc.tensor.matmul(
                                        st8["ot"][:, pi0 * P:], lhsT=v_r[:, pkb, :],
                                        rhs=pputs[:, pi0 * P:],
                                        start=(pkb == 0), stop=False)
                                st8["prev"] = (kb, puts, i0)
                            return t

                        def mk_fin(h, st8, qc=qc, kmax=kmax):
                            def t():
                                pkb, pputs, pi0 = st8["prev"]
                                nc.tensor.matmul(
                                    st8["ot"][:, pi0 * P:], lhsT=v_r[:, pkb, :],
                                    rhs=pputs[:, pi0 * P:],
                                    start=(pkb == 0), stop=True)
                                csl = slice(qc * 512, (qc + 1) * 512)
                                # sigmoid(g) = 1/(1+exp(-g)) here in P3 where
                                # the exp table set is already resident
                                eng = sgpool.tile([P, 512], F32, tag="eng",
                                                   name="eng")
                                nc.scalar.activation(eng[:], graw[:, h, csl],
                                                     AF.Exp, scale=-1.0)
                                en1 = sgpool.tile([P, 512], F32, tag="en1",
                                                   name="en1")
                                nc.vector.tensor_scalar_add(en1[:], eng[:], 1.0)
                                sig = sgpool.tile([P, 512], F32, tag="sig",
                                                   name="sig")
                                nc.vector.reciprocal_approx_fast(sig[:], en1[:])
                                nc.vector.tensor_mul(
                                    gated_r[:, h, csl], st8["ot"][:], sig[:])
                            return t

                        for kb in range(kmax + 1):
                            thunks.append(mk_kb(h, kb, st8))
                        thunks.append(mk_fin(h, st8))
                    for sb in range(4 * qc, 4 * qc + 4):
                        cst = {"cpo": None}

                        def mk_wo(sb, dc, cst):
                            def t():
                                if cst["cpo"] is None:
                                    cst["cpo"] = copool.tile(
                                        [P, NSC, 512], F32, name="cpo")
                                pso = ptp.tile([P, 512], F32, name="putp")
                                for h in range(HPC):
                                    nc.tensor.matmul(
                                        pso[:],
                                        lhsT=gated_r[:, h, sb * P:(sb + 1) * P],
                                        rhs=wo_sb[:, h, dc * 512:(dc + 1) * 512],
                                        start=(h == 0), stop=(h == HPC - 1))
                                evac_copy(cst["cpo"][:, dc, :], pso[:])
                                if dc == NSC - 1:
                                    nc.sync.dma_start(
                                        out[sb * P:(sb + 1) * P, :],
                                        cst["cpo"][:].rearrange("p dc m -> p (dc m)"))
                            return t

                        for dc in range(NSC):
                            thunks.append(mk_wo(sb, dc, cst))
                    return thunks

                # the last sq slice's norm+rope stages become qc=0's fill:
                # they interleave with the first softmax chains instead of
                # serializing ahead of them in the engine queues
                fill = norm_stages(pend, scp=tlp, rrp=tlr, pnp=ptp)
                pend = []
                for qc in range(NSC):
                    pu_l = {}
                    d_l = {}
                    n0 = len(fill)
                    done = 0
                    for ci, (qbi, h) in enumerate(
                            [(b, hh_) for b in range(4) for hh_ in range(HPC)]):
                        softmax_chain(qc, qbi, h, pu_l, d_l)
                        # drain a proportional slice of the previous q-chunk's
                        # AV/Wo work between softmax chains
                        tgt = n0 * (ci + 1) // 8
                        while done < tgt:
                            fill[done]()
                            done += 1
                    for t in fill[done:]:
                        t()
                    fill = avwo_thunks(qc, pu_l, d_l)
                # tail: last q-chunk's AV + Wo runs PE-dense, no filler needed
                for t in fill:
                    t()
            _otp_cm.__exit__(None, None, None)
            _ptp_cm.__exit__(None, None, None)
            _nxr_cm.__exit__(None, None, None)
            _cs_cm.__exit__(None, None, None)
            _tlr_cm.__exit__(None, None, None)
            _tlp_cm.__exit__(None, None, None)

    nc.compile()
    return nc


def _host_inputs(hidden_states, cos, sin, Wq, Wk, Wv, Wo, q_norm_w, k_norm_w):
    import ml_dtypes
    hs = np.asarray(hidden_states, dtype=np.float32).reshape(S, DM)
    # [dm, s] -> [p, kc, s] so every DMA is contiguous per partition
    hsT_l = np.ascontiguousarray(hs.T.reshape(KCH, P, S).transpose(1, 0, 2))
    hsT32 = _round_fp32r(hsT_l)
    hsT16 = hsT_l.astype(np.float16)
    cosT = np.ascontiguousarray(np.asarray(cos, np.float32).T)
    sinT = np.ascontiguousarray(np.asarray(sin, np.float32).T)
    sinTs = sinT.copy()
    sinTs[:D // 2] = -sinTs[:D // 2]     # sign of rotate-half folded into sin
    identb = np.eye(P, dtype=np.float32).astype(ml_dtypes.bfloat16)
    oner = np.ones((1, P), np.float32)
    triu = np.triu(np.full((P, P), -1e9, np.float32), 1)
    qw = np.asarray(q_norm_w, np.float32).reshape(D)
    kw = np.asarray(k_norm_w, np.float32).reshape(D)
    # fold norm weights into the projection columns; sumsq then needs 1/w^2
    Wq = np.asarray(Wq, np.float32).copy()
    for h in range(16):
        Wq[:, h * 2 * D:h * 2 * D + D] *= qw[None, :]
    Wk = np.asarray(Wk, np.float32) * np.tile(kw, 4)[None, :]
    Wv = np.asarray(Wv, np.float32)
    Wo = np.asarray(Wo, np.float32)
    wi2q = (1.0 / (qw * qw)).reshape(P, 1).astype(np.float16)
    wi2k = (1.0 / (kw * kw)).reshape(P, 1).astype(np.float16)

    def wlayout(w, f16):
        # [dm, cols] -> [p, kc, cols] contiguous
        wl = np.ascontiguousarray(w.reshape(KCH, P, w.shape[1]).transpose(1, 0, 2))
        return wl.astype(np.float16) if f16 else _round_fp32r(wl)

    maps = []
    for c in range(NCORES):
        heads = [2 * c, 2 * c + 1]
        g = c // 2
        wq_c = np.concatenate([Wq[:, h * 2 * D:h * 2 * D + D] for h in heads], axis=1)
        wg_c = np.concatenate([Wq[:, h * 2 * D + D:(h + 1) * 2 * D] for h in heads], axis=1)
        wo_c = np.ascontiguousarray(
            Wo[c * 2 * D:(c + 1) * 2 * D, :].reshape(HPC, P, DM).transpose(1, 0, 2)
        ).astype(np.float16)
        maps.append({
            "hsT": hsT32,
            "hsT16": hsT16,
            "wq": wlayout(wq_c, False),
            "wk": wlayout(Wk[:, g * D:(g + 1) * D], False),
            "wg": wlayout(wg_c, True),
            "wv": wlayout(Wv[:, g * D:(g + 1) * D], True),
            "wo": wo_c,
            "cosT": cosT, "sinTs": sinTs,
            "wi2q": wi2q, "wi2k": wi2k,
            "identb": identb, "oner": oner, "triu": triu,
        })
    return maps


def kernel(**inputs):
    from concourse.bass_utils import run_bass_kernel_spmd

    if "nc" not in _cache:
        _cache["nc"] = _build_nc()
    nc = _cache["nc"]
    maps = _host_inputs(
        inputs["hidden_states"], inputs["cos"], inputs["sin"],
        inputs["Wq"], inputs["Wk"], inputs["Wv"], inputs["Wo"],
        inputs["q_norm_w"], inputs["k_norm_w"])
    res = run_bass_kernel_spmd(nc, maps, list(range(NCORES)))
    total = np.zeros((S, DM), np.float64)
    for r in res.results:
        total += r["out"].astype(np.float64)
    return total.astype(np.float32).reshape(1, S, DM)


# revision 57
# speedup vs baseline: 1.1313x; 1.1313x over previous
"""Qwen3-style GQA attention (B=1, S=2048, DM=2048, H=16, KV=4, D=128) on 8 TRN2 cores.

Sharding: tensor-parallel over heads. Core c computes Q heads {2c, 2c+1} and
KV head c//2 end-to-end, then a partial output hs_part = gated_local @ Wo_rows.
Host sums the 8 partials.

Precision scheme: the q/k score path runs f32r (12-bit host-prerounded
operands, fp32_mode=HIGH 2-pass matmuls) -- fp16 operands measured 2.6e-2
rel err vs the 2e-2 gate, so the score path keeps f32r. The gate/V
projections and the Wo matmul run fp16 single-pass (1 cycle/row, their noise
is linear and ~3e-4); hs streams in both f32r (sync queue) and fp16 (scalar
queue) copies. Post-softmax probabilities, diag(1/Z), and V run bf16.

Activation-table discipline: P1's scalar engine only ever runs Sqrt (rms),
P3's only Exp (softmax + gate sigmoid as 1/(1+exp(-x)) with the reciprocal
on the vector engine; raw gate is stored in P1 and the sigmoid deferred to
P3). Two ACT_TABLE_LOADs total instead of per-iteration thrash.

P1 structure: one fused projection pass streams hsT chunks once; the first
kc chunk of each weight + hs DMAs ahead of everything so matmuls start ~5us
in. RMSNorm + RoPE jobs are deferred one sq slice and emitted as 4
stage-thunks interleaved into the next slice's projection k4 loop, so every
engine's in-order queue sees 3 independent jobs per stage (chains pipeline
instead of serializing). Rotate-half is an exact partition-offset SBUF DMA
on the gpsimd queue; the three sumsq matmuls of a batch share one PSUM bank
at 32-row offsets.

P3 structure: dual-pass softmax per (h, qb) -- a bf16 max pre-pass feeds
exp's bias so the f32r score pass goes matmul->exp with no reduce between
(PSUM banks free immediately). P^T for AV comes from bf16 matmuls against
diag(1/Z), fusing normalization into the transpose. Cross-phase software
pipelining: chunk qc's PuT/AV/Wo work is emitted as thunks drained between
chunk qc+1's softmax chains, keeping the PE warm through the reduce/exp
latency; per-kb AV matmuls are emitted one kb late so the in-order PE queue
never stalls on a puts copy. PSUM evacuations alternate vector/scalar.

The last sq slice's norm+rope stages are deferred into P3 entirely: their
scratch lives in pools that outlive P1 (manual __enter__/__exit__), their
sumsq/broadcast matmuls run in the early-opened ptp PSUM pool, and the four
stage thunks seed qc=0's fill queue so they interleave with the first
softmax chains. This keeps the P1 pool-close barrier (which gates P3's
first PSUM matmuls on the last reader of every closed pool) free of the
norm tail. Note co pool must stay at bufs=2: a single cpo buffer serializes
every Wo output DMA against the next block's copies (+40us measured).

Measured (8 cores, this problem): 332-343us, rel err 1.15e-2.
Rejected variants (measured): all-fp16 q/k (2.6e-2 err), single-pass
PSUM-resident softmax (bank-holding serializes chains, +30us), XBAR
dma_start_transpose for P^T (transfer time stalls AV, +40us) and for V
(+7us, collides with the hh16 stream on the scalar DGE queue), Ln+Exp rms
(table-set thrash, 2.5us/job), single-pass softmax at qc<=1 only (+8us),
fusing the rope add into persistent q_hi (+50us, cause unknown).
"""

import numpy as np

S = 2048
DM = 2048
D = 128
HPC = 2           # q heads per core
NCORES = 8
SCALING = float(D) ** 0.5
EPS = 1e-6
P = 128
KCH = DM // P     # 16 contraction chunks for projections
NQB = S // P      # 16 q blocks
NSC = S // 512    # 4 seq chunks of 512

_cache = {}


def _round_fp32r(x):
    x = np.ascontiguousarray(x, dtype=np.float32)
    b = x.view(np.uint32).astype(np.uint64)
    lsb = (b >> 12) & 1
    r = (b + 0x7FF + lsb) & 0xFFFFF000
    return r.astype(np.uint32).view(np.float32)


def _build_nc():
    import concourse.tile as tile
    from concourse import bacc, mybir

    F32 = mybir.dt.float32
    F32R = mybir.dt.float32r
    F16 = mybir.dt.float16
    BF16 = mybir.dt.bfloat16
    AF = mybir.ActivationFunctionType
    from concourse.alu_op_type import AluOpType as ALU
    AX = mybir.AxisListType.X

    nc = bacc.Bacc(None, target_bir_lowering=False, debug=False)

    with nc.allow_low_precision(reason="f32r/fp16/bf16 operands are a "
                                "deliberate precision/speed tradeoff"), \
         tile.TileContext(nc) as tc:
        with tc.tile_pool(name="dram", bufs=1, space="DRAM") as dram:
            hsT = dram.tile([P, KCH, S], F32R, kind="ExternalInput", name="hsT", uniquify=False)
            hsT16 = dram.tile([P, KCH, S], F16, kind="ExternalInput", name="hsT16", uniquify=False)
            wq = dram.tile([P, KCH, HPC * P], F32R, kind="ExternalInput", name="wq", uniquify=False)
            wk = dram.tile([P, KCH, P], F32R, kind="ExternalInput", name="wk", uniquify=False)
            wg = dram.tile([P, KCH, HPC * P], F16, kind="ExternalInput", name="wg", uniquify=False)
            wv = dram.tile([P, KCH, P], F16, kind="ExternalInput", name="wv", uniquify=False)
            wo = dram.tile([P, HPC, DM], F16, kind="ExternalInput", name="wo", uniquify=False)
            cosT = dram.tile([P, S], F32, kind="ExternalInput", name="cosT", uniquify=False)
            sinTs = dram.tile([P, S], F32, kind="ExternalInput", name="sinTs", uniquify=False)
            wi2q = dram.tile([P, 1], F16, kind="ExternalInput", name="wi2q", uniquify=False)
            wi2k = dram.tile([P, 1], F16, kind="ExternalInput", name="wi2k", uniquify=False)
            identb = dram.tile([P, P], BF16, kind="ExternalInput", name="identb", uniquify=False)
            oner = dram.tile([1, P], F32, kind="ExternalInput", name="oner", uniquify=False)
            triu = dram.tile([P, P], F32, kind="ExternalInput", name="triu", uniquify=False)
            out = dram.tile([S, DM], F32, kind="ExternalOutput", name="out", uniquify=False)

        # persistent SBUF (whole kernel)
        with tc.tile_pool(name="persist", bufs=1) as pers:
            wi2q_sb = pers.tile([P, 1], F16)
            dum = pers.tile([P, 64], F16)
            wi2k_sb = pers.tile([P, 1], F16)
            identb_sb = pers.tile([P, P], BF16)
            oner_sb = pers.tile([1, P], F32)
            triu_sb = pers.tile([P, P], F32)
            eps_sb = pers.tile([4, 1], F32)
            k_hi = pers.tile([P, S], F32R)
            k_hb = pers.tile([P, S], BF16)
            q_hi = pers.tile([P, HPC, S], F32R)
            q_hb = pers.tile([P, HPC, S], BF16)
            graw = pers.tile([P, HPC, S], F32)      # raw gate (sigmoid in P3)
            v_r = pers.tile([P, NQB, P], BF16)      # V untransposed (s-major blocks)
            gated_r = pers.tile([P, HPC, S], F16)

            nc.gpsimd.memset(eps_sb[:], EPS)
            nc.gpsimd.memset(dum[:], 0.0)

            # tail-scratch outlives P1 so the last norm batch's tiles don't
            # extend the P1 pool-close barrier that gates P3's first matmuls
            _tlp_cm = tc.tile_pool(name="tlp", bufs=10)
            _tlr_cm = tc.tile_pool(name="tlr", bufs=1)
            _cs_cm = tc.tile_pool(name="cs", bufs=2)
            _nxr_cm = tc.tile_pool(name="nxr", bufs=5)
            tlp = _tlp_cm.__enter__()
            tlr = _tlr_cm.__enter__()
            cspool = _cs_cm.__enter__()
            nxr = _nxr_cm.__enter__()

            # ====== P1 (fused): all projections + norm + rope + splits ======
            with (
                tc.tile_pool(name="wts", bufs=1) as wpool,
                tc.tile_pool(name="hs1", bufs=2) as hspool,
                tc.tile_pool(name="nsc", bufs=9) as nsc,
                tc.tile_pool(name="rr1", bufs=1) as rr1,
                tc.tile_pool(name="vts", bufs=1) as vtp,
                tc.tile_pool(name="pqk", bufs=1, space="PSUM") as pqk,
                tc.tile_pool(name="pnm", bufs=1, space="PSUM") as pnm,
            ):
                # HAM warm-up: ~5us of dummy matmuls spin the PE during the
                # initial DMA wait; nothing reads the target bank
                dum_ps = pnm.tile([P, 512], F32, tag="psb", name="psb")
                for _ in range(100):
                    nc.tensor.matmul(dum_ps[0:64, 0:64], lhsT=dum[:], rhs=dum[:],
                                     start=True, stop=True)

                wq_sb = wpool.tile([P, KCH, HPC * P], F32R)
                wk_sb = wpool.tile([P, KCH, P], F32R)
                wg_sb = wpool.tile([P, KCH, HPC * P], F16)
                wv_sb = wpool.tile([P, KCH, P], F16)

                pend = []

                def norm_stages(jobs, scp=None, rrp=None, pnp=None):
                    # 4 stage-thunks for up to 3 norm+rope jobs, interleaved
                    # into the next sq's projection emission so each engine's
                    # in-order queue pipelines jobs instead of serializing
                    # behind one job's cross-engine latency chain
                    st = {}
                    sp = scp or nsc
                    rp = rrp or rr1

                    def s1():
                        if pnp is not None:
                            ps3 = pnp.tile([P, 512], F32, name="putp")
                        else:
                            ps3 = pnm.tile([P, 512], F32, tag="ps1", name="ps3")
                        sqfs = []
                        for j, (xr, wvec, xhi, xhb, cos_t, sin_t) in enumerate(jobs):
                            sqf = sp.tile([P, 512], F16, tag="scr", name="sqf")
                            nc.vector.tensor_mul(sqf[:], xr[:], xr[:])
                            sqfs.append(sqf)
                        for j, (xr, wvec, xhi, xhb, cos_t, sin_t) in enumerate(jobs):
                            nc.tensor.matmul(ps3[32 * j:32 * j + 1, :], lhsT=wvec[:],
                                             rhs=sqfs[j][:], start=True, stop=True)
                        st["ps3"] = ps3

                    def s2():
                        rrs = []
                        for j in range(len(jobs)):
                            sqv = rp.tile([1, 512], F32, tag="sqv%d" % (0 if pnp is not None else j), name="sqv")
                            nc.scalar.activation(sqv[:], st["ps3"][32 * j:32 * j + 1, :],
                                                 AF.Sqrt, scale=1.0 / D,
                                                 bias=eps_sb[0:1, :])
                            rr = rp.tile([1, 512], F32, tag="rr%d" % (0 if pnp is not None else j), name="rr")
                            nc.vector.reciprocal_approx_fast(rr[:], sqv[:])
                            rrs.append(rr)
                        st["rrs"] = rrs

                    def s3():
                        xns, rots = [], []
                        for j, (xr, wvec, xhi, xhb, cos_t, sin_t) in enumerate(jobs):
                            if pnp is not None:
                                psb = pnp.tile([P, 512], F32, name="putp")
                            else:
                                psb = pnm.tile([P, 512], F32, tag="psb", name="psb")
                            nc.tensor.matmul(psb[:], lhsT=oner_sb[:],
                                             rhs=st["rrs"][j][:], start=True, stop=True)
                            xn = sp.tile([P, 512], F32, tag="scr", name="xn")
                            nc.vector.tensor_mul(xn[:], xr[:], psb[:])
                            rot = sp.tile([P, 512], F32, tag="scr", name="rot")
                            nc.gpsimd.dma_start(rot[0:64, :], xn[64:128, :])
                            nc.gpsimd.dma_start(rot[64:128, :], xn[0:64, :])
                            xns.append(xn)
                            rots.append(rot)
                        st["xns"], st["rots"] = xns, rots

                    def s4():
                        t2s, t1s = [], []
                        for j, (xr, wvec, xhi, xhb, cos_t, sin_t) in enumerate(jobs):
                            t2 = sp.tile([P, 512], F32, tag="scr", name="t2")
                            nc.vector.tensor_mul(t2[:], st["rots"][j][:], sin_t[:])
                            t1 = sp.tile([P, 512], F32, tag="scr", name="t1")
                            nc.gpsimd.tensor_mul(t1[:], st["xns"][j][:], cos_t[:])
                            t2s.append(t2)
                            t1s.append(t1)
                        for j, (xr, wvec, xhi, xhb, cos_t, sin_t) in enumerate(jobs):
                            xf = sp.tile([P, 512], F32, tag="scr", name="xf")
                            nc.vector.tensor_add(xf[:], t1s[j][:], t2s[j][:])
                            nc.vector.tensor_copy(xhi, xf[:])
                            nc.scalar.copy(xhb, xhi.bitcast(F32))

                    return [s1, s2, s3, s4]

                for sq in range(NSC):
                    stages = norm_stages(pend[:3]) if sq > 0 else None
                    del pend[:3]
                    s0 = sq * 512
                    sl = slice(s0, s0 + 512)
                    cos_t = cspool.tile([P, 512], F32, tag="cos", name="cos_t")
                    sin_t = cspool.tile([P, 512], F32, tag="sin", name="sin_t")
                    if sq > 0:
                        nc.scalar.dma_start(cos_t[:], cosT[:, sl])
                        nc.scalar.dma_start(sin_t[:], sinTs[:, sl])
                    ps_q0 = pqk.tile([P, 512], F32, tag="psq0", name="ps_q0")
                    ps_q1 = pqk.tile([P, 512], F32, tag="psq1", name="ps_q1")
                    ps_k = pqk.tile([P, 512], F32, tag="psk", name="ps_k")
                    ps_g0 = pqk.tile([P, 512], F32, tag="psg0", name="ps_g0")
                    ps_g1 = pqk.tile([P, 512], F32, tag="psg1", name="ps_g1")
                    ps_v = pqk.tile([P, 512], F32, tag="psv", name="ps_v")
                    for k4 in range(4):
                        hh = hspool.tile([P, 4, 512], F32R, tag="hh", name="hh")
                        hh16 = hspool.tile([P, 4, 512], F16, tag="hh16", name="hh16")
                        if sq == 0 and k4 == 0:
                            # critical first chunk: kc=0 of each weight + the
                            # first hs slice, so matmuls start ~5us in; the
                            # persistent constants ride behind them
                            for (dst, srcw) in (
                                (wq_sb, wq), (wk_sb, wk), (wg_sb, wg), (wv_sb, wv),
                            ):
                                nc.sync.dma_start(dst[:, 0:1, :], srcw[:, 0:1, :])
                            nc.sync.dma_start(hh[:, 0:1, :], hsT[:, 0:1, sl])
                            nc.scalar.dma_start(hh16[:, 0:1, :], hsT16[:, 0:1, sl])
                            nc.sync.dma_start(wi2q_sb[:], wi2q[:])
                            nc.sync.dma_start(wi2k_sb[:], wi2k[:])
                            nc.sync.dma_start(identb_sb[:], identb[:])
                            nc.sync.dma_start(oner_sb[:], oner[:])
                            nc.sync.dma_start(triu_sb[:], triu[:])
                            for (dst, srcw) in (
                                (wq_sb, wq), (wk_sb, wk), (wg_sb, wg), (wv_sb, wv),
                            ):
                                nc.sync.dma_start(dst[:, 1:4, :], srcw[:, 1:4, :])
                            nc.sync.dma_start(hh[:, 1:4, :], hsT[:, 1:4, sl])
                            nc.scalar.dma_start(hh16[:, 1:4, :], hsT16[:, 1:4, sl])
                            nc.scalar.dma_start(cos_t[:], cosT[:, sl])
                            nc.scalar.dma_start(sin_t[:], sinTs[:, sl])
                        else:
                            if sq == 0:
                                ksl = slice(k4 * 4, k4 * 4 + 4)
                                for (dst, srcw) in (
                                    (wq_sb, wq), (wk_sb, wk), (wg_sb, wg), (wv_sb, wv),
                                ):
                                    nc.sync.dma_start(dst[:, ksl, :], srcw[:, ksl, :])
                            # fp16 hs copy rides the scalar queue in parallel
                            # with the f32r copy on the sync queue
                            nc.scalar.dma_start(hh16[:], hsT16[:, k4 * 4:k4 * 4 + 4, sl])
                            nc.sync.dma_start(hh[:], hsT[:, k4 * 4:k4 * 4 + 4, sl])
                        for kci in range(4):
                            kc = k4 * 4 + kci
                            st = kc == 0
                            sp = kc == KCH - 1
                            hx = hh[:, kci, :]
                            hx16 = hh16[:, kci, :]
                            nc.tensor.matmul(ps_q0[:], lhsT=wq_sb[:, kc, 0:P],
                                             rhs=hx, start=st, stop=sp)
                            nc.tensor.matmul(ps_q1[:], lhsT=wq_sb[:, kc, P:2 * P],
                                             rhs=hx, start=st, stop=sp)
                            nc.tensor.matmul(ps_k[:], lhsT=wk_sb[:, kc, :],
                                             rhs=hx, start=st, stop=sp)
                            nc.tensor.matmul(ps_g0[:], lhsT=wg_sb[:, kc, 0:P],
                                             rhs=hx16, start=st, stop=sp)
                            nc.tensor.matmul(ps_g1[:], lhsT=wg_sb[:, kc, P:2 * P],
                                             rhs=hx16, start=st, stop=sp)
                            nc.tensor.matmul(ps_v[:], lhsT=wv_sb[:, kc, :],
                                             rhs=hx16, start=st, stop=sp)
                        if stages is not None:
                            stages[k4]()
                    # gate: store raw; sigmoid runs in P3 where exp's table
                    # set is already loaded (copy is in every table set)
                    nc.any.tensor_copy(graw[:, 0, sl], ps_g0[:])
                    nc.any.tensor_copy(graw[:, 1, sl], ps_g1[:])
                    # V: bf16 copy + transpose into s-major blocks
                    vt = vtp.tile([P, 512], BF16, tag="vt", name="vt")
                    nc.any.tensor_copy(vt[:], ps_v[:])
                    for j in range(4):
                        pst = pqk.tile([P, P], BF16, tag="psg0", name="pst")
                        nc.tensor.transpose(pst[:], vt[:, j * P:(j + 1) * P], identb_sb[:])
                        nc.any.tensor_copy(v_r[:, sq * 4 + j, :], pst[:])
                    # Q/K: copy raw projections out now (frees PSUM); the
                    # norm/rope chain is deferred one sq iteration so the next
                    # projection block hides its PE matmuls' input latency
                    for (psd, wvec, xhi, xhb) in (
                        (ps_q0, wi2q_sb, q_hi[:, 0, sl], q_hb[:, 0, sl]),
                        (ps_q1, wi2q_sb, q_hi[:, 1, sl], q_hb[:, 1, sl]),
                        (ps_k, wi2k_sb, k_hi[:, sl], k_hb[:, sl]),
                    ):
                        xr = nxr.tile([P, 512], F32, tag="xr", name="xr")
                        nc.any.tensor_copy(xr[:], psd[:])
                        pend.append((xr, wvec, xhi, xhb, cos_t, sin_t))
                # sq3's norm jobs are deferred into P3's fill queue

            # ====== P3: attention; dual-pass softmax, cross-qc pipeline ======
            _ptp_cm = tc.tile_pool(name="ptp", bufs=2, space="PSUM")
            _otp_cm = tc.tile_pool(name="otp", bufs=1, space="PSUM")
            ptp = _ptp_cm.__enter__()
            otp = _otp_cm.__enter__()
            with (
                tc.tile_pool(name="mxp", bufs=2, space="PSUM") as mxp,
                tc.tile_pool(name="scb", bufs=3, space="PSUM") as scb,
                tc.tile_pool(name="pu", bufs=10) as pupool,
                tc.tile_pool(name="dd", bufs=10) as ddpool,
                tc.tile_pool(name="sm", bufs=16) as smpool,
                tc.tile_pool(name="sgp", bufs=1) as sgpool,
                tc.tile_pool(name="pts", bufs=2) as ptspool,
                tc.tile_pool(name="wop", bufs=1) as wopool,
                tc.tile_pool(name="co", bufs=2) as copool,
            ):
                wo_sb = wopool.tile([P, HPC, DM], F16)
                nc.sync.dma_start(wo_sb[:], wo[:])

                rot3 = [0]

                def evac_copy(dst, src):
                    # alternate PSUM evacuations between vector and scalar
                    # (gpsimd has no PSUM access)
                    r = rot3[0] = (rot3[0] + 1) % 2
                    if r == 0:
                        nc.vector.tensor_copy(dst, src)
                    else:
                        nc.scalar.copy(dst, src)

                def softmax_chain(qc, qbi, h, pu_l, d_l):
                    qb = 4 * qc + qbi
                    r = qb % 4
                    qsl = slice(qb * P, (qb + 1) * P)
                    nful = qc
                    # --- bf16 max pre-pass: approximate row max ---
                    mparts = smpool.tile([P, 8], F32, tag="mp", name="mparts")
                    for kc in range(nful + 1):
                        w = 512 if kc < nful else (r + 1) * P
                        ksl = slice(kc * 512, kc * 512 + w)
                        mx = mxp.tile([P, 512], F32, name="mx")
                        nc.tensor.matmul(mx[:, :w], lhsT=q_hb[:, h, qsl],
                                         rhs=k_hb[:, ksl], start=True, stop=True)
                        if kc == nful:
                            nc.vector.tensor_add(
                                mx[:, r * P:(r + 1) * P],
                                mx[:, r * P:(r + 1) * P], triu_sb[:])
                        nc.vector.tensor_reduce(
                            mparts[:, kc:kc + 1], mx[:, :w], axis=AX, op=ALU.max)
                    negm = smpool.tile([P, 1], F32, tag="negm", name="negm")
                    nc.vector.tensor_reduce(
                        negm[:], mparts[:, :nful + 1], axis=AX, op=ALU.max,
                        negate=True)
                    bias_t = smpool.tile([P, 1], F32, tag="bias", name="bias_t")
                    nc.vector.tensor_scalar_mul(bias_t[:], negm[:], SCALING)
                    # --- f32r scores; exp immediately, no reduce between ---
                    pu = pupool.tile([P, S], BF16, tag="pu", name="pu")
                    zparts = smpool.tile([P, 8], F32, tag="zp", name="zparts")
                    for kc in range(nful + 1):
                        w = 512 if kc < nful else (r + 1) * P
                        ksl = slice(kc * 512, kc * 512 + w)
                        ps = scb.tile([P, 512], F32, name="ps")
                        nc.tensor.matmul(
                            ps[:, :w], lhsT=q_hi[:, h, qsl], rhs=k_hi[:, ksl],
                            start=True, stop=True)
                        if kc == nful:
                            nc.vector.tensor_add(
                                ps[:, r * P:(r + 1) * P],
                                ps[:, r * P:(r + 1) * P], triu_sb[:])
                        nc.scalar.activation(
                            pu[:, kc * 512:kc * 512 + w], ps[:, :w], AF.Exp,
                            scale=SCALING, bias=bias_t[:],
                            accum_out=zparts[:, kc:kc + 1])
                    zsum = smpool.tile([P, 1], F32, tag="zs", name="zsum")
                    nc.vector.tensor_reduce(
                        zsum[:], zparts[:, :nful + 1], axis=AX, op=ALU.add)
                    rz = smpool.tile([P, 1], F32, tag="rz", name="rz")
                    nc.vector.reciprocal_approx_fast(rz[:], zsum[:])
                    dmat = ddpool.tile([P, P], BF16, tag="dm", name="dmat")
                    nc.vector.tensor_scalar_mul(dmat[:], identb_sb[:], rz[:])
                    pu_l[(h, qb)] = pu
                    d_l[(h, qb)] = dmat

                def avwo_thunks(qc, pu_l, d_l):
                    # PuT+AV per (h, kb) with the AV matmul deferred one kb so
                    # the in-order PE queue never waits on a puts copy; then
                    # the Wo partials + output DMA for this q-chunk's rows.
                    kmax = 4 * qc + 3
                    thunks = []
                    for h in range(HPC):
                        st8 = {"prev": None, "ot": None}

                        def mk_kb(h, kb, st8):
                            def t():
                                if st8["ot"] is None:
                                    st8["ot"] = otp.tile([P, 512], F32, name="ot_ps")
                                putp = ptp.tile([P, 512], F32, name="putp")
                                i0 = max(kb - 4 * qc, 0)
                                for j in range(i0, 4):
                                    qb = 4 * qc + j
                                    nc.tensor.matmul(
                                        putp[:, j * P:(j + 1) * P],
                                        lhsT=pu_l[(h, qb)][:, kb * P:(kb + 1) * P],
                                        rhs=d_l[(h, qb)][:],
                                        start=True, stop=True)
                                puts = ptspool.tile([P, 512], BF16, name="puts")
                                evac_copy(puts[:, i0 * P:], putp[:, i0 * P:])
                                if st8["prev"] is not None:
                                    pkb, pputs, pi0 = st8["prev"]
                                    nc.tensor.matmul(
                                        st8["ot"][:, pi0 * P:], lhsT=v_r[:, pkb, :],
                                        rhs=pputs[:, pi0 * P:],
                                        start=(pkb == 0), stop=False)
                                st8["prev"] = (kb, puts, i0)
                            return t

                        def mk_fin(h, st8, qc=qc, kmax=kmax):
                            def t():
                                pkb, pputs, pi0 = st8["prev"]
                                nc.tensor.matmul(
                                    st8["ot"][:, pi0 * P:], lhsT=v_r[:, pkb, :],
                                    rhs=pputs[:, pi0 * P:],
                                    start=(pkb == 0), stop=True)
                                csl = slice(qc * 512, (qc + 1) * 512)
                                # sigmoid(g) = 1/(1+exp(-g)) here in P3 where
                                # the exp table set is already resident
                                eng = sgpool.tile([P, 512], F32, tag="eng",
                                                   name="eng")
                                nc.scalar.activation(eng[:], graw[:, h, csl],
                                                     AF.Exp, scale=-1.0)
                                en1 = sgpool.tile([P, 512], F32, tag="en1",
                                                   name="en1")
                                nc.vector.tensor_scalar_add(en1[:], eng[:], 1.0)
                                sig = sgpool.tile([P, 512], F32, tag="sig",
                                                   name="sig")
                                nc.vector.reciprocal_approx_fast(sig[:], en1[:])
                                nc.vector.tensor_mul(
                                    gated_r[:, h, csl], st8["ot"][:], sig[:])
                            return t

                        for kb in range(kmax + 1):
                            thunks.append(mk_kb(h, kb, st8))
                        thunks.append(mk_fin(h, st8))
                    for sb in range(4 * qc, 4 * qc + 4):
                        cst = {"cpo": None}

                        def mk_wo(sb, dc, cst):
                            def t():
                                if cst["cpo"] is None:
                                    cst["cpo"] = copool.tile(
                                        [P, NSC, 512], F32, name="cpo")
                                pso = ptp.tile([P, 512], F32, name="putp")
                                for h in range(HPC):
                                    nc.tensor.matmul(
                                        pso[:],
                                        lhsT=gated_r[:, h, sb * P:(sb + 1) * P],
                                        rhs=wo_sb[:, h, dc * 512:(dc + 1) * 512],
                                        start=(h == 0), stop=(h == HPC - 1))
                                evac_copy(cst["cpo"][:, dc, :], pso[:])
                                if dc == NSC - 1:
                                    nc.sync.dma_start(
                                        out[sb * P:(sb + 1) * P, :],
                                        cst["cpo"][:].rearrange("p dc m -> p (dc m)"))
                            return t

                        for dc in range(NSC):
                            thunks.append(mk_wo(sb, dc, cst))
                    return thunks

                # the last sq slice's norm+rope stages become qc=0's fill:
                # they interleave with the first softmax chains instead of
                # serializing ahead of them in the engine queues
                fill = norm_stages(pend, scp=tlp, rrp=tlr, pnp=ptp)
                pend = []
                for qc in range(NSC):
                    pu_l = {}
                    d_l = {}
                    n0 = len(fill)
                    done = 0
                    for ci, (qbi, h) in enumerate(
                            [(b, hh_) for b in range(4) for hh_ in range(HPC)]):
                        softmax_chain(qc, qbi, h, pu_l, d_l)
                        # drain a proportional slice of the previous q-chunk's
                        # AV/Wo work between softmax chains
                        tgt = n0 * (ci + 1) // 8
                        while done < tgt:
                            fill[done]()
                            done += 1
                    for t in fill[done:]:
                        t()
                    fill = avwo_thunks(qc, pu_l, d_l)
                # tail: last q-chunk's AV + Wo runs PE-dense, no filler needed
                for t in fill:
                    t()
            _otp_cm.__exit__(None, None, None)
            _ptp_cm.__exit__(None, None, None)
            _nxr_cm.__exit__(None, None, None)
            _cs_cm.__exit__(None, None, None)
            _tlr_cm.__exit__(None, None, None)
            _tlp_cm.__exit__(None, None, None)

    nc.compile()
    return nc


def _host_inputs(hidden_states, cos, sin, Wq, Wk, Wv, Wo, q_norm_w, k_norm_w):
    import ml_dtypes
    hs = np.asarray(hidden_states, dtype=np.float32).reshape(S, DM)
    # [dm, s] -> [p, kc, s] so every DMA is contiguous per partition
    hsT_l = np.ascontiguousarray(hs.T.reshape(KCH, P, S).transpose(1, 0, 2))
    hsT32 = _round_fp32r(hsT_l)
    hsT16 = hsT_l.astype(np.float16)
    cosT = np.ascontiguousarray(np.asarray(cos, np.float32).T)
    sinT = np.ascontiguousarray(np.asarray(sin, np.float32).T)
    sinTs = sinT.copy()
    sinTs[:D // 2] = -sinTs[:D // 2]     # sign of rotate-half folded into sin
    identb = np.eye(P, dtype=np.float32).astype(ml_dtypes.bfloat16)
    oner = np.ones((1, P), np.float32)
    triu = np.triu(np.full((P, P), -1e9, np.float32), 1)
    qw = np.asarray(q_norm_w, np.float32).reshape(D)
    kw = np.asarray(k_norm_w, np.float32).reshape(D)
    # fold norm weights into the projection columns; sumsq then needs 1/w^2
    Wq = np.asarray(Wq, np.float32).copy()
    for h in range(16):
        Wq[:, h * 2 * D:h * 2 * D + D] *= qw[None, :]
    Wk = np.asarray(Wk, np.float32) * np.tile(kw, 4)[None, :]
    Wv = np.asarray(Wv, np.float32)
    Wo = np.asarray(Wo, np.float32)
    wi2q = (1.0 / (qw * qw)).reshape(P, 1).astype(np.float16)
    wi2k = (1.0 / (kw * kw)).reshape(P, 1).astype(np.float16)

    def wlayout(w, f16):
        # [dm, cols] -> [p, kc, cols] contiguous
        wl = np.ascontiguousarray(w.reshape(KCH, P, w.shape[1]).transpose(1, 0, 2))
        return wl.astype(np.float16) if f16 else _round_fp32r(wl)

    maps = []
    for c in range(NCORES):
        heads = [2 * c, 2 * c + 1]
        g = c // 2
        wq_c = np.concatenate([Wq[:, h * 2 * D:h * 2 * D + D] for h in heads], axis=1)
        wg_c = np.concatenate([Wq[:, h * 2 * D + D:(h + 1) * 2 * D] for h in heads], axis=1)
        wo_c = np.ascontiguousarray(
            Wo[c * 2 * D:(c + 1) * 2 * D, :].reshape(HPC, P, DM).transpose(1, 0, 2)
        ).astype(np.float16)
        maps.append({
            "hsT": hsT32,
            "hsT16": hsT16,
            "wq": wlayout(wq_c, False),
            "wk": wlayout(Wk[:, g * D:(g + 1) * D], False),
            "wg": wlayout(wg_c, True),
            "wv": wlayout(Wv[:, g * D:(g + 1) * D], True),
            "wo": wo_c,
            "cosT": cosT, "sinTs": sinTs,
            "wi2q": wi2q, "wi2k": wi2k,
            "identb": identb, "oner": oner, "triu": triu,
        })
    return maps


def kernel(**inputs):
    from concourse.bass_utils import run_bass_kernel_spmd

    if "nc" not in _cache:
        _cache["nc"] = _build_nc()
    nc = _cache["nc"]
    maps = _host_inputs(
        inputs["hidden_states"], inputs["cos"], inputs["sin"],
        inputs["Wq"], inputs["Wk"], inputs["Wv"], inputs["Wo"],
        inputs["q_norm_w"], inputs["k_norm_w"])
    res = run_bass_kernel_spmd(nc, maps, list(range(NCORES)))
    total = np.zeros((S, DM), np.float64)
    for r in res.results:
        total += r["out"].astype(np.float64)
    return total.astype(np.float32).reshape(1, S, DM)


# revision 58
# speedup vs baseline: 1.1465x; 1.0134x over previous
"""Qwen3-style GQA attention (B=1, S=2048, DM=2048, H=16, KV=4, D=128) on 8 TRN2 cores.

Sharding: tensor-parallel over heads. Core c computes Q heads {2c, 2c+1} and
KV head c//2 end-to-end, then a partial output hs_part = gated_local @ Wo_rows.
Host sums the 8 partials.

Precision scheme: the q/k score path runs f32r (12-bit host-prerounded
operands, fp32_mode=HIGH 2-pass matmuls) -- fp16 operands measured 2.6e-2
rel err vs the 2e-2 gate, so the score path keeps f32r. The gate/V
projections and the Wo matmul run fp16 single-pass (1 cycle/row, their noise
is linear and ~3e-4); hs streams in both f32r (sync queue) and fp16 (scalar
queue) copies. Post-softmax probabilities, diag(1/Z), and V run bf16.

Activation-table discipline: P1's scalar engine only ever runs Sqrt (rms),
P3's only Exp (softmax + gate sigmoid as 1/(1+exp(-x)) with the reciprocal
on the vector engine; raw gate is stored in P1 and the sigmoid deferred to
P3). Two ACT_TABLE_LOADs total instead of per-iteration thrash.

P1 structure: one fused projection pass streams hsT chunks once; the first
kc chunk of each weight + hs DMAs ahead of everything so matmuls start ~5us
in. RMSNorm + RoPE jobs are deferred one sq slice and emitted as 4
stage-thunks interleaved into the next slice's projection k4 loop, so every
engine's in-order queue sees 3 independent jobs per stage (chains pipeline
instead of serializing). Rotate-half is an exact partition-offset SBUF DMA
on the gpsimd queue; the three sumsq matmuls of a batch share one PSUM bank
at 32-row offsets.

P3 structure: dual-pass softmax per (h, qb) -- a bf16 max pre-pass feeds
exp's bias so the f32r score pass goes matmul->exp with no reduce between
(PSUM banks free immediately). P^T for AV comes from bf16 matmuls against
diag(1/Z), fusing normalization into the transpose. Cross-phase software
pipelining: chunk qc's PuT/AV/Wo work is emitted as thunks drained between
chunk qc+1's softmax chains, keeping the PE warm through the reduce/exp
latency; per-kb AV matmuls are emitted one kb late so the in-order PE queue
never stalls on a puts copy. PSUM evacuations alternate vector/scalar.

The last sq slice's norm+rope stages are deferred into P3 entirely: their
scratch lives in pools that outlive P1 (manual __enter__/__exit__), their
sumsq/broadcast matmuls run in the early-opened ptp PSUM pool, and the four
stage thunks seed qc=0's fill queue so they interleave with the first
softmax chains. This keeps the P1 pool-close barrier (which gates P3's
first PSUM matmuls on the last reader of every closed pool) free of the
norm tail. Note co pool must stay at bufs=2: a single cpo buffer serializes
every Wo output DMA against the next block's copies (+40us measured).

Measured (8 cores, this problem): 332-343us, rel err 1.15e-2.
Rejected variants (measured): all-fp16 q/k (2.6e-2 err), single-pass
PSUM-resident softmax (bank-holding serializes chains, +30us), XBAR
dma_start_transpose for P^T (transfer time stalls AV, +40us) and for V
(+7us, collides with the hh16 stream on the scalar DGE queue), Ln+Exp rms
(table-set thrash, 2.5us/job), single-pass softmax at qc<=1 only (+8us),
fusing the rope add into persistent q_hi (+50us, cause unknown).
"""

import numpy as np

S = 2048
DM = 2048
D = 128
HPC = 2           # q heads per core
NCORES = 8
SCALING = float(D) ** 0.5
EPS = 1e-6
P = 128
KCH = DM // P     # 16 contraction chunks for projections
NQB = S // P      # 16 q blocks
NSC = S // 512    # 4 seq chunks of 512

_cache = {}


def _round_fp32r(x):
    x = np.ascontiguousarray(x, dtype=np.float32)
    b = x.view(np.uint32).astype(np.uint64)
    lsb = (b >> 12) & 1
    r = (b + 0x7FF + lsb) & 0xFFFFF000
    return r.astype(np.uint32).view(np.float32)


def _build_nc():
    import concourse.tile as tile
    from concourse import bacc, mybir

    F32 = mybir.dt.float32
    F32R = mybir.dt.float32r
    F16 = mybir.dt.float16
    BF16 = mybir.dt.bfloat16
    AF = mybir.ActivationFunctionType
    from concourse.alu_op_type import AluOpType as ALU
    AX = mybir.AxisListType.X

    nc = bacc.Bacc(None, target_bir_lowering=False, debug=False)

    with nc.allow_low_precision(reason="f32r/fp16/bf16 operands are a "
                                "deliberate precision/speed tradeoff"), \
         tile.TileContext(nc) as tc:
        with tc.tile_pool(name="dram", bufs=1, space="DRAM") as dram:
            hsT = dram.tile([P, KCH, S], F32R, kind="ExternalInput", name="hsT", uniquify=False)
            hsT16 = dram.tile([P, KCH, S], F16, kind="ExternalInput", name="hsT16", uniquify=False)
            wq = dram.tile([P, KCH, HPC * P], F32R, kind="ExternalInput", name="wq", uniquify=False)
            wk = dram.tile([P, KCH, P], F32R, kind="ExternalInput", name="wk", uniquify=False)
            wg = dram.tile([P, KCH, HPC * P], F16, kind="ExternalInput", name="wg", uniquify=False)
            wv = dram.tile([P, KCH, P], F16, kind="ExternalInput", name="wv", uniquify=False)
            wo = dram.tile([P, HPC, DM], F16, kind="ExternalInput", name="wo", uniquify=False)
            cosT = dram.tile([P, S], F32, kind="ExternalInput", name="cosT", uniquify=False)
            sinTs = dram.tile([P, S], F32, kind="ExternalInput", name="sinTs", uniquify=False)
            wi2q = dram.tile([P, 1], F16, kind="ExternalInput", name="wi2q", uniquify=False)
            wi2k = dram.tile([P, 1], F16, kind="ExternalInput", name="wi2k", uniquify=False)
            identb = dram.tile([P, P], BF16, kind="ExternalInput", name="identb", uniquify=False)
            oner = dram.tile([1, P], F32, kind="ExternalInput", name="oner", uniquify=False)
            triu = dram.tile([P, P], F32, kind="ExternalInput", name="triu", uniquify=False)
            out = dram.tile([S, DM], F32, kind="ExternalOutput", name="out", uniquify=False)

        # persistent SBUF (whole kernel)
        with tc.tile_pool(name="persist", bufs=1) as pers:
            wi2q_sb = pers.tile([P, 1], F16)
            dum = pers.tile([P, 64], F16)
            wi2k_sb = pers.tile([P, 1], F16)
            identb_sb = pers.tile([P, P], BF16)
            oner_sb = pers.tile([1, P], F32)
            triu_sb = pers.tile([P, P], F32)
            eps_sb = pers.tile([4, 1], F32)
            k_hi = pers.tile([P, S], F32R)
            k_hb = pers.tile([P, S], BF16)
            q_hi = pers.tile([P, HPC, S], F32R)
            q_hb = pers.tile([P, HPC, S], BF16)
            graw = pers.tile([P, HPC, S], F32)      # raw gate (sigmoid in P3)
            v_r = pers.tile([P, NQB, P], BF16)      # V untransposed (s-major blocks)
            gated_r = pers.tile([P, HPC, S], F16)

            nc.gpsimd.memset(eps_sb[:], EPS)
            nc.gpsimd.memset(dum[:], 0.0)

            # tail-scratch outlives P1 so the last norm batch's tiles don't
            # extend the P1 pool-close barrier that gates P3's first matmuls
            _tlp_cm = tc.tile_pool(name="tlp", bufs=10)
            _tlr_cm = tc.tile_pool(name="tlr", bufs=1)
            _cs_cm = tc.tile_pool(name="cs", bufs=2)
            _nxr_cm = tc.tile_pool(name="nxr", bufs=5)
            tlp = _tlp_cm.__enter__()
            tlr = _tlr_cm.__enter__()
            cspool = _cs_cm.__enter__()
            nxr = _nxr_cm.__enter__()

            # ====== P1 (fused): all projections + norm + rope + splits ======
            with (
                tc.tile_pool(name="wts", bufs=1) as wpool,
                tc.tile_pool(name="hs1", bufs=2) as hspool,
                tc.tile_pool(name="nsc", bufs=9) as nsc,
                tc.tile_pool(name="rr1", bufs=1) as rr1,
                tc.tile_pool(name="vts", bufs=1) as vtp,
                tc.tile_pool(name="pqk", bufs=1, space="PSUM") as pqk,
                tc.tile_pool(name="pnm", bufs=1, space="PSUM") as pnm,
            ):
                # HAM warm-up: ~5us of dummy matmuls spin the PE during the
                # initial DMA wait; nothing reads the target bank
                dum_ps = pnm.tile([P, 512], F32, tag="psb", name="psb")
                for _ in range(100):
                    nc.tensor.matmul(dum_ps[0:64, 0:64], lhsT=dum[:], rhs=dum[:],
                                     start=True, stop=True)

                wq_sb = wpool.tile([P, KCH, HPC * P], F32R)
                wk_sb = wpool.tile([P, KCH, P], F32R)
                wg_sb = wpool.tile([P, KCH, HPC * P], F16)
                wv_sb = wpool.tile([P, KCH, P], F16)

                pend = []

                def norm_stages(jobs, scp=None, rrp=None, pnp=None):
                    # 4 stage-thunks for up to 3 norm+rope jobs, interleaved
                    # into the next sq's projection emission so each engine's
                    # in-order queue pipelines jobs instead of serializing
                    # behind one job's cross-engine latency chain
                    st = {}
                    sp = scp or nsc
                    rp = rrp or rr1

                    def s1():
                        if pnp is not None:
                            ps3 = pnp.tile([P, 512], F32, name="putp")
                        else:
                            ps3 = pnm.tile([P, 512], F32, tag="ps1", name="ps3")
                        sqfs = []
                        for j, (xr, wvec, xhi, xhb, cos_t, sin_t) in enumerate(jobs):
                            sqf = sp.tile([P, 512], F16, tag="scr", name="sqf")
                            nc.vector.tensor_mul(sqf[:], xr[:], xr[:])
                            sqfs.append(sqf)
                        for j, (xr, wvec, xhi, xhb, cos_t, sin_t) in enumerate(jobs):
                            nc.tensor.matmul(ps3[32 * j:32 * j + 1, :], lhsT=wvec[:],
                                             rhs=sqfs[j][:], start=True, stop=True)
                        st["ps3"] = ps3

                    def s2():
                        rrs = []
                        for j in range(len(jobs)):
                            sqv = rp.tile([1, 512], F32, tag="sqv%d" % (0 if pnp is not None else j), name="sqv")
                            nc.scalar.activation(sqv[:], st["ps3"][32 * j:32 * j + 1, :],
                                                 AF.Sqrt, scale=1.0 / D,
                                                 bias=eps_sb[0:1, :])
                            rr = rp.tile([1, 512], F32, tag="rr%d" % (0 if pnp is not None else j), name="rr")
                            nc.vector.reciprocal_approx_fast(rr[:], sqv[:])
                            rrs.append(rr)
                        st["rrs"] = rrs

                    def s3():
                        xns, rots = [], []
                        for j, (xr, wvec, xhi, xhb, cos_t, sin_t) in enumerate(jobs):
                            if pnp is not None:
                                psb = pnp.tile([P, 512], F32, name="putp")
                            else:
                                psb = pnm.tile([P, 512], F32, tag="psb", name="psb")
                            nc.tensor.matmul(psb[:], lhsT=oner_sb[:],
                                             rhs=st["rrs"][j][:], start=True, stop=True)
                            xn = sp.tile([P, 512], F32, tag="scr", name="xn")
                            nc.vector.tensor_mul(xn[:], xr[:], psb[:])
                            rot = sp.tile([P, 512], F32, tag="scr", name="rot")
                            nc.gpsimd.dma_start(rot[0:64, :], xn[64:128, :])
                            nc.gpsimd.dma_start(rot[64:128, :], xn[0:64, :])
                            xns.append(xn)
                            rots.append(rot)
                        st["xns"], st["rots"] = xns, rots

                    def s4():
                        t2s, t1s = [], []
                        for j, (xr, wvec, xhi, xhb, cos_t, sin_t) in enumerate(jobs):
                            t2 = sp.tile([P, 512], F32, tag="scr", name="t2")
                            nc.vector.tensor_mul(t2[:], st["rots"][j][:], sin_t[:])
                            t1 = sp.tile([P, 512], F32, tag="scr", name="t1")
                            nc.gpsimd.tensor_mul(t1[:], st["xns"][j][:], cos_t[:])
                            t2s.append(t2)
                            t1s.append(t1)
                        for j, (xr, wvec, xhi, xhb, cos_t, sin_t) in enumerate(jobs):
                            xf = sp.tile([P, 512], F32, tag="scr", name="xf")
                            nc.vector.tensor_add(xf[:], t1s[j][:], t2s[j][:])
                            nc.vector.tensor_copy(xhi, xf[:])
                            nc.scalar.copy(xhb, xhi.bitcast(F32))

                    return [s1, s2, s3, s4]

                for sq in range(NSC):
                    stages = norm_stages(pend[:3]) if sq > 0 else None
                    del pend[:3]
                    s0 = sq * 512
                    sl = slice(s0, s0 + 512)
                    cos_t = cspool.tile([P, 512], F32, tag="cos", name="cos_t")
                    sin_t = cspool.tile([P, 512], F32, tag="sin", name="sin_t")
                    if sq > 0:
                        nc.scalar.dma_start(cos_t[:], cosT[:, sl])
                        nc.scalar.dma_start(sin_t[:], sinTs[:, sl])
                    ps_q0 = pqk.tile([P, 512], F32, tag="psq0", name="ps_q0")
                    ps_q1 = pqk.tile([P, 512], F32, tag="psq1", name="ps_q1")
                    ps_k = pqk.tile([P, 512], F32, tag="psk", name="ps_k")
                    ps_g0 = pqk.tile([P, 512], F32, tag="psg0", name="ps_g0")
                    ps_g1 = pqk.tile([P, 512], F32, tag="psg1", name="ps_g1")
                    ps_v = pqk.tile([P, 512], F32, tag="psv", name="ps_v")
                    for k4 in range(4):
                        hh = hspool.tile([P, 4, 512], F32R, tag="hh", name="hh")
                        hh16 = hspool.tile([P, 4, 512], F16, tag="hh16", name="hh16")
                        if sq == 0 and k4 == 0:
                            # critical first chunk: kc=0 of each weight + the
                            # first hs slice, so matmuls start ~5us in; the
                            # persistent constants ride behind them
                            for (dst, srcw) in (
                                (wq_sb, wq), (wk_sb, wk), (wg_sb, wg), (wv_sb, wv),
                            ):
                                nc.sync.dma_start(dst[:, 0:1, :], srcw[:, 0:1, :])
                            nc.sync.dma_start(hh[:, 0:1, :], hsT[:, 0:1, sl])
                            nc.scalar.dma_start(hh16[:, 0:1, :], hsT16[:, 0:1, sl])
                            nc.sync.dma_start(wi2q_sb[:], wi2q[:])
                            nc.sync.dma_start(wi2k_sb[:], wi2k[:])
                            nc.sync.dma_start(identb_sb[:], identb[:])
                            nc.sync.dma_start(oner_sb[:], oner[:])
                            nc.sync.dma_start(triu_sb[:], triu[:])
                            for (dst, srcw) in (
                                (wq_sb, wq), (wk_sb, wk), (wg_sb, wg), (wv_sb, wv),
                            ):
                                nc.sync.dma_start(dst[:, 1:4, :], srcw[:, 1:4, :])
                            nc.sync.dma_start(hh[:, 1:4, :], hsT[:, 1:4, sl])
                            nc.scalar.dma_start(hh16[:, 1:4, :], hsT16[:, 1:4, sl])
                            nc.scalar.dma_start(cos_t[:], cosT[:, sl])
                            nc.scalar.dma_start(sin_t[:], sinTs[:, sl])
                        else:
                            if sq == 0:
                                ksl = slice(k4 * 4, k4 * 4 + 4)
                                for (dst, srcw) in (
                                    (wq_sb, wq), (wk_sb, wk), (wg_sb, wg), (wv_sb, wv),
                                ):
                                    nc.sync.dma_start(dst[:, ksl, :], srcw[:, ksl, :])
                            # fp16 hs copy rides the scalar queue in parallel
                            # with the f32r copy on the sync queue
                            nc.scalar.dma_start(hh16[:], hsT16[:, k4 * 4:k4 * 4 + 4, sl])
                            nc.sync.dma_start(hh[:], hsT[:, k4 * 4:k4 * 4 + 4, sl])
                        for kci in range(4):
                            kc = k4 * 4 + kci
                            st = kc == 0
                            sp = kc == KCH - 1
                            hx = hh[:, kci, :]
                            hx16 = hh16[:, kci, :]
                            nc.tensor.matmul(ps_q0[:], lhsT=wq_sb[:, kc, 0:P],
                                             rhs=hx, start=st, stop=sp)
                            nc.tensor.matmul(ps_q1[:], lhsT=wq_sb[:, kc, P:2 * P],
                                             rhs=hx, start=st, stop=sp)
                            nc.tensor.matmul(ps_k[:], lhsT=wk_sb[:, kc, :],
                                             rhs=hx, start=st, stop=sp)
                            nc.tensor.matmul(ps_g0[:], lhsT=wg_sb[:, kc, 0:P],
                                             rhs=hx16, start=st, stop=sp)
                            nc.tensor.matmul(ps_g1[:], lhsT=wg_sb[:, kc, P:2 * P],
                                             rhs=hx16, start=st, stop=sp)
                            nc.tensor.matmul(ps_v[:], lhsT=wv_sb[:, kc, :],
                                             rhs=hx16, start=st, stop=sp)
                        if stages is not None:
                            stages[k4]()
                    # gate: store raw; sigmoid runs in P3 where exp's table
                    # set is already loaded (copy is in every table set)
                    nc.any.tensor_copy(graw[:, 0, sl], ps_g0[:])
                    nc.any.tensor_copy(graw[:, 1, sl], ps_g1[:])
                    # V: bf16 copy + transpose into s-major blocks
                    vt = vtp.tile([P, 512], BF16, tag="vt", name="vt")
                    nc.any.tensor_copy(vt[:], ps_v[:])
                    for j in range(4):
                        pst = pqk.tile([P, P], BF16, tag="psg0", name="pst")
                        nc.tensor.transpose(pst[:], vt[:, j * P:(j + 1) * P], identb_sb[:])
                        nc.any.tensor_copy(v_r[:, sq * 4 + j, :], pst[:])
                    # Q/K: copy raw projections out now (frees PSUM); the
                    # norm/rope chain is deferred one sq iteration so the next
                    # projection block hides its PE matmuls' input latency
                    for (psd, wvec, xhi, xhb) in (
                        (ps_q0, wi2q_sb, q_hi[:, 0, sl], q_hb[:, 0, sl]),
                        (ps_q1, wi2q_sb, q_hi[:, 1, sl], q_hb[:, 1, sl]),
                        (ps_k, wi2k_sb, k_hi[:, sl], k_hb[:, sl]),
                    ):
                        xr = nxr.tile([P, 512], F32, tag="xr", name="xr")
                        nc.any.tensor_copy(xr[:], psd[:])
                        pend.append((xr, wvec, xhi, xhb, cos_t, sin_t))
                # sq3's norm jobs are deferred into P3's fill queue

            # ====== P3: attention; dual-pass softmax, cross-qc pipeline ======
            _ptp_cm = tc.tile_pool(name="ptp", bufs=2, space="PSUM")
            _otp_cm = tc.tile_pool(name="otp", bufs=1, space="PSUM")
            ptp = _ptp_cm.__enter__()
            otp = _otp_cm.__enter__()
            with (
                tc.tile_pool(name="mxp", bufs=2, space="PSUM") as mxp,
                tc.tile_pool(name="scb", bufs=3, space="PSUM") as scb,
                tc.tile_pool(name="pu", bufs=10) as pupool,
                tc.tile_pool(name="dd", bufs=10) as ddpool,
                tc.tile_pool(name="sm", bufs=16) as smpool,
                tc.tile_pool(name="sgp", bufs=1) as sgpool,
                tc.tile_pool(name="pts", bufs=2) as ptspool,
                tc.tile_pool(name="wop", bufs=1) as wopool,
                tc.tile_pool(name="co", bufs=2) as copool,
            ):
                wo_sb = wopool.tile([P, HPC, DM], F16)
                nc.sync.dma_start(wo_sb[:], wo[:])

                rot3 = [0]

                def evac_copy(dst, src):
                    # alternate PSUM evacuations between vector and scalar
                    # (gpsimd has no PSUM access)
                    r = rot3[0] = (rot3[0] + 1) % 2
                    if r == 0:
                        nc.vector.tensor_copy(dst, src)
                    else:
                        nc.scalar.copy(dst, src)

                def softmax_chain(qc, qbi, h, pu_l, d_l):
                    qb = 4 * qc + qbi
                    r = qb % 4
                    qsl = slice(qb * P, (qb + 1) * P)
                    nful = qc
                    # --- bf16 max pre-pass: approximate row max ---
                    mparts = smpool.tile([P, 8], F32, tag="mp", name="mparts")
                    for kc in range(nful + 1):
                        w = 512 if kc < nful else (r + 1) * P
                        ksl = slice(kc * 512, kc * 512 + w)
                        mx = mxp.tile([P, 512], F32, name="mx")
                        nc.tensor.matmul(mx[:, :w], lhsT=q_hb[:, h, qsl],
                                         rhs=k_hb[:, ksl], start=True, stop=True)
                        if kc == nful:
                            nc.vector.tensor_add(
                                mx[:, r * P:(r + 1) * P],
                                mx[:, r * P:(r + 1) * P], triu_sb[:])
                        nc.vector.tensor_reduce(
                            mparts[:, kc:kc + 1], mx[:, :w], axis=AX, op=ALU.max)
                    negm = smpool.tile([P, 1], F32, tag="negm", name="negm")
                    nc.vector.tensor_reduce(
                        negm[:], mparts[:, :nful + 1], axis=AX, op=ALU.max,
                        negate=True)
                    bias_t = smpool.tile([P, 1], F32, tag="bias", name="bias_t")
                    nc.vector.tensor_scalar_mul(bias_t[:], negm[:], SCALING)
                    # --- f32r scores; exp immediately, no reduce between ---
                    pu = pupool.tile([P, S], BF16, tag="pu", name="pu")
                    zparts = smpool.tile([P, 8], F32, tag="zp", name="zparts")
                    for kc in range(nful + 1):
                        w = 512 if kc < nful else (r + 1) * P
                        ksl = slice(kc * 512, kc * 512 + w)
                        ps = scb.tile([P, 512], F32, name="ps")
                        nc.tensor.matmul(
                            ps[:, :w], lhsT=q_hi[:, h, qsl], rhs=k_hi[:, ksl],
                            start=True, stop=True)
                        if kc == nful:
                            nc.vector.tensor_add(
                                ps[:, r * P:(r + 1) * P],
                                ps[:, r * P:(r + 1) * P], triu_sb[:])
                        nc.scalar.activation(
                            pu[:, kc * 512:kc * 512 + w], ps[:, :w], AF.Exp,
                            scale=SCALING, bias=bias_t[:],
                            accum_out=zparts[:, kc:kc + 1])
                    zsum = smpool.tile([P, 1], F32, tag="zs", name="zsum")
                    nc.vector.tensor_reduce(
                        zsum[:], zparts[:, :nful + 1], axis=AX, op=ALU.add)
                    rz = smpool.tile([P, 1], F32, tag="rz", name="rz")
                    nc.vector.reciprocal_approx_fast(rz[:], zsum[:])
                    dmat = ddpool.tile([P, P], BF16, tag="dm", name="dmat")
                    nc.vector.tensor_scalar_mul(dmat[:], identb_sb[:], rz[:])
                    pu_l[(h, qb)] = pu
                    d_l[(h, qb)] = dmat

                def avwo_thunks(qc, pu_l, d_l):
                    # PuT+AV per (h, kb) with the AV matmul deferred one kb so
                    # the in-order PE queue never waits on a puts copy; then
                    # the Wo partials + output DMA for this q-chunk's rows.
                    kmax = 4 * qc + 3
                    thunks = []
                    for h in range(HPC):
                        st8 = {"prev": None, "ot": None}

                        def mk_kb(h, kb, st8):
                            def t():
                                if st8["ot"] is None:
                                    st8["ot"] = otp.tile([P, 512], F32, name="ot_ps")
                                putp = ptp.tile([P, 512], F32, name="putp")
                                i0 = max(kb - 4 * qc, 0)
                                for j in range(i0, 4):
                                    qb = 4 * qc + j
                                    nc.tensor.matmul(
                                        putp[:, j * P:(j + 1) * P],
                                        lhsT=pu_l[(h, qb)][:, kb * P:(kb + 1) * P],
                                        rhs=d_l[(h, qb)][:],
                                        start=True, stop=True)
                                puts = ptspool.tile([P, 512], BF16, name="puts")
                                evac_copy(puts[:, i0 * P:], putp[:, i0 * P:])
                                if st8["prev"] is not None:
                                    pkb, pputs, pi0 = st8["prev"]
                                    nc.tensor.matmul(
                                        st8["ot"][:, pi0 * P:], lhsT=v_r[:, pkb, :],
                                        rhs=pputs[:, pi0 * P:],
                                        start=(pkb == 0), stop=False)
                                st8["prev"] = (kb, puts, i0)
                            return t

                        def mk_fin(h, st8, qc=qc, kmax=kmax):
                            def t():
                                pkb, pputs, pi0 = st8["prev"]
                                nc.tensor.matmul(
                                    st8["ot"][:, pi0 * P:], lhsT=v_r[:, pkb, :],
                                    rhs=pputs[:, pi0 * P:],
                                    start=(pkb == 0), stop=True)
                                csl = slice(qc * 512, (qc + 1) * 512)
                                # sigmoid(g) = 1/(1+exp(-g)) here in P3 where
                                # the exp table set is already resident
                                eng = sgpool.tile([P, 512], F32, tag="eng",
                                                   name="eng")
                                nc.scalar.activation(eng[:], graw[:, h, csl],
                                                     AF.Exp, scale=-1.0)
                                en1 = sgpool.tile([P, 512], F32, tag="en1",
                                                   name="en1")
                                nc.vector.tensor_scalar_add(en1[:], eng[:], 1.0)
                                sig = sgpool.tile([P, 512], F32, tag="sig",
                                                   name="sig")
                                nc.vector.reciprocal_approx_fast(sig[:], en1[:])
                                nc.vector.tensor_mul(
                                    gated_r[:, h, csl], st8["ot"][:], sig[:])
                            return t

                        for kb in range(kmax + 1):
                            thunks.append(mk_kb(h, kb, st8))
                        thunks.append(mk_fin(h, st8))
                    for sb in range(4 * qc, 4 * qc + 4):
                        cst = {"cpo": None}

                        def mk_wo(sb, dc, cst):
                            def t():
                                if cst["cpo"] is None:
                                    cst["cpo"] = copool.tile(
                                        [P, NSC, 512], F32, name="cpo")
                                pso = ptp.tile([P, 512], F32, name="putp")
                                for h in range(HPC):
                                    nc.tensor.matmul(
                                        pso[:],
                                        lhsT=gated_r[:, h, sb * P:(sb + 1) * P],
                                        rhs=wo_sb[:, h, dc * 512:(dc + 1) * 512],
                                        start=(h == 0), stop=(h == HPC - 1))
                                evac_copy(cst["cpo"][:, dc, :], pso[:])
                                if dc == NSC - 1:
                                    nc.sync.dma_start(
                                        out[sb * P:(sb + 1) * P, :],
                                        cst["cpo"][:].rearrange("p dc m -> p (dc m)"))
                            return t

                        for dc in range(NSC):
                            thunks.append(mk_wo(sb, dc, cst))
                    return thunks

                # the last sq slice's norm+rope stages become qc=0's fill:
                # they interleave with the first softmax chains instead of
                # serializing ahead of them in the engine queues; dummy
                # warm-keeper thunks (unread writes into the mx rotation)
                # spread between them keep the HAM clock warm through the
                # seam's idle PE windows
                def mk_warm():
                    def t():
                        mxw = mxp.tile([P, 512], F32, name="mx")
                        for _ in range(2):
                            nc.tensor.matmul(mxw[0:64, 0:64], lhsT=dum[:],
                                             rhs=dum[:], start=True, stop=True)
                    return t
                fill = []
                for s in norm_stages(pend, scp=tlp, rrp=tlr, pnp=ptp):
                    fill.append(s)
                    fill.extend(mk_warm() for _ in range(3))
                pend = []
                for qc in range(NSC):
                    pu_l = {}
                    d_l = {}
                    n0 = len(fill)
                    done = 0
                    for ci, (qbi, h) in enumerate(
                            [(b, hh_) for b in range(4) for hh_ in range(HPC)]):
                        softmax_chain(qc, qbi, h, pu_l, d_l)
                        # drain a proportional slice of the previous q-chunk's
                        # AV/Wo work between softmax chains
                        tgt = n0 * (ci + 1) // 8
                        while done < tgt:
                            fill[done]()
                            done += 1
                    for t in fill[done:]:
                        t()
                    fill = avwo_thunks(qc, pu_l, d_l)
                # tail: last q-chunk's AV + Wo runs PE-dense, no filler needed
                for t in fill:
                    t()
            _otp_cm.__exit__(None, None, None)
            _ptp_cm.__exit__(None, None, None)
            _nxr_cm.__exit__(None, None, None)
            _cs_cm.__exit__(None, None, None)
            _tlr_cm.__exit__(None, None, None)
            _tlp_cm.__exit__(None, None, None)

    nc.compile()
    return nc


def _host_inputs(hidden_states, cos, sin, Wq, Wk, Wv, Wo, q_norm_w, k_norm_w):
    import ml_dtypes
    hs = np.asarray(hidden_states, dtype=np.float32).reshape(S, DM)
    # [dm, s] -> [p, kc, s] so every DMA is contiguous per partition
    hsT_l = np.ascontiguousarray(hs.T.reshape(KCH, P, S).transpose(1, 0, 2))
    hsT32 = _round_fp32r(hsT_l)
    hsT16 = hsT_l.astype(np.float16)
    cosT = np.ascontiguousarray(np.asarray(cos, np.float32).T)
    sinT = np.ascontiguousarray(np.asarray(sin, np.float32).T)
    sinTs = sinT.copy()
    sinTs[:D // 2] = -sinTs[:D // 2]     # sign of rotate-half folded into sin
    identb = np.eye(P, dtype=np.float32).astype(ml_dtypes.bfloat16)
    oner = np.ones((1, P), np.float32)
    triu = np.triu(np.full((P, P), -1e9, np.float32), 1)
    qw = np.asarray(q_norm_w, np.float32).reshape(D)
    kw = np.asarray(k_norm_w, np.float32).reshape(D)
    # fold norm weights into the projection columns; sumsq then needs 1/w^2
    Wq = np.asarray(Wq, np.float32).copy()
    for h in range(16):
        Wq[:, h * 2 * D:h * 2 * D + D] *= qw[None, :]
    Wk = np.asarray(Wk, np.float32) * np.tile(kw, 4)[None, :]
    Wv = np.asarray(Wv, np.float32)
    Wo = np.asarray(Wo, np.float32)
    wi2q = (1.0 / (qw * qw)).reshape(P, 1).astype(np.float16)
    wi2k = (1.0 / (kw * kw)).reshape(P, 1).astype(np.float16)

    def wlayout(w, f16):
        # [dm, cols] -> [p, kc, cols] contiguous
        wl = np.ascontiguousarray(w.reshape(KCH, P, w.shape[1]).transpose(1, 0, 2))
        return wl.astype(np.float16) if f16 else _round_fp32r(wl)

    maps = []
    for c in range(NCORES):
        heads = [2 * c, 2 * c + 1]
        g = c // 2
        wq_c = np.concatenate([Wq[:, h * 2 * D:h * 2 * D + D] for h in heads], axis=1)
        wg_c = np.concatenate([Wq[:, h * 2 * D + D:(h + 1) * 2 * D] for h in heads], axis=1)
        wo_c = np.ascontiguousarray(
            Wo[c * 2 * D:(c + 1) * 2 * D, :].reshape(HPC, P, DM).transpose(1, 0, 2)
        ).astype(np.float16)
        maps.append({
            "hsT": hsT32,
            "hsT16": hsT16,
            "wq": wlayout(wq_c, False),
            "wk": wlayout(Wk[:, g * D:(g + 1) * D], False),
            "wg": wlayout(wg_c, True),
            "wv": wlayout(Wv[:, g * D:(g + 1) * D], True),
            "wo": wo_c,
            "cosT": cosT, "sinTs": sinTs,
            "wi2q": wi2q, "wi2k": wi2k,
            "identb": identb, "oner": oner, "triu": triu,
        })
    return maps


def kernel(**inputs):
    from concourse.bass_utils import run_bass_kernel_spmd

    if "nc" not in _cache:
        _cache["nc"] = _build_nc()
    nc = _cache["nc"]
    maps = _host_inputs(
        inputs["hidden_states"], inputs["cos"], inputs["sin"],
        inputs["Wq"], inputs["Wk"], inputs["Wv"], inputs["Wo"],
        inputs["q_norm_w"], inputs["k_norm_w"])
    res = run_bass_kernel_spmd(nc, maps, list(range(NCORES)))
    total = np.zeros((S, DM), np.float64)
    for r in res.results:
        total += r["out"].astype(np.float64)
    return total.astype(np.float32).reshape(1, S, DM)
